# revision 1
# baseline (speedup 1.0000x reference)
"""BatchAdaptiveConv2d Trainium2 kernel (8 NeuronCores, data parallel).

Math: out[b] = conv2d_same(x[b], W * wadapt[b, ci]) + bias * badapt[b]
 - wadapt[b] = cat(cond[b], lpe[b]) @ wa_w.T + wa_b    (per-sample Cin scale)
 - badapt[b] = cat(cond[b], lpe[b]) @ ba_w.T + ba_b    (per-sample Cout bias scale)
Since the conv is linear, the per-sample kernel modulation is folded into the
(small) weights on-chip; the conv itself is one shared 3x3, Cin=Cout=32.

Per-core plan (2 samples each):
 - x tile [128=(q,ci), 34, 256] f32: 4 row-quarters of 32 rows (+1 halo row
   each side) stacked on partition groups; SAME pad via AP trims + memsets.
 - 16-way PE 32x32 sub-tile packing: tile_position=(32q, 32j); row group q =
   quarter, col group j = 2-row pixel block; 9 tap matmuls (K=32ci, M=32co,
   N<=512, dtype float32r) accumulate into psum[32j:32j+32, bank q].
 - PSUM wave [128, 4*2*256] -> ScalarE activation Copy with per-partition
   bias (= bias*badapt replicated x4) -> SBUF -> DMA to HBM.
"""

import numpy as np

B, CIN, COUT, KK, H, W = 16, 32, 32, 3, 256, 256
EMB = 256
NCORES = 8
SB = B // NCORES  # samples per core

_CACHE = {}


def _build_nc(
    n_samples=SB,
    n_tiles=2,
    n_waves=4,
    n_q=4,
    n_j=4,
    n_taps=9,
    reps_loop=1,
    skip_conv=False,
):
    import concourse.bacc as bacc
    import concourse.bass as bass
    import concourse.mybir as mybir
    from concourse.tile import TileContext

    f32 = mybir.dt.float32
    bf16 = mybir.dt.bfloat16
    mult = mybir.AluOpType.mult
    add = mybir.AluOpType.add
    Identity = mybir.ActivationFunctionType.Identity

    nc = bacc.Bacc()

    x_d = nc.declare_dram_parameter("x", [SB, CIN, H, W], f32, isOutput=False)
    cond_d = nc.declare_dram_parameter("condition", [SB, EMB], f32, isOutput=False)
    lpe_d = nc.declare_dram_parameter(
        "layer_pos_embedding", [SB, EMB], f32, isOutput=False
    )
    w_d = nc.declare_dram_parameter("weights", [CIN, COUT, KK, KK], f32, isOutput=False)
    bias_d = nc.declare_dram_parameter("bias", [COUT], f32, isOutput=False)
    waw_d = nc.declare_dram_parameter("wa_w", [CIN, 2 * EMB], f32, isOutput=False)
    wab_d = nc.declare_dram_parameter("wa_b", [CIN], f32, isOutput=False)
    baw_d = nc.declare_dram_parameter("ba_w", [COUT, 2 * EMB], f32, isOutput=False)
    bab_d = nc.declare_dram_parameter("ba_b", [COUT], f32, isOutput=False)
    out_d = nc.declare_dram_parameter("out", [SB, COUT, H, W], f32, isOutput=True)

    def dram_ap(handle, offset, dims):
        a = handle[:]
        return bass.AP(tensor=a.tensor, offset=offset, ap=[list(d) for d in dims])

    HW_ = H * W  # 65536, per-channel plane
    SOFF = CIN * HW_  # per-sample x offset

    with TileContext(nc) as tc:
        with (
            tc.tile_pool(name="const", bufs=1) as const,
            tc.tile_pool(name="xin", bufs=3) as xpool,
            tc.tile_pool(name="ostg", bufs=3) as opool,
            tc.tile_pool(name="ps", bufs=2, space="PSUM") as ppool,
        ):
            # ---- constants, replicated x4 across quarter partition groups ----
            # (SBUF AP dim0 must be the partition dim -> one DMA per group)
            w_all = const.tile([128, COUT, KK, KK], f32, tag="w_all")
            waw_all = const.tile([128, 2 * EMB], f32, tag="waw_all")
            baw_all = const.tile([128, 2 * EMB], f32, tag="baw_all")
            bias_all = const.tile([128, 1], f32, tag="bias_all")
            wab_all = const.tile([128, 1], f32, tag="wab_all")
            bab_all = const.tile([128, 1], f32, tag="bab_all")
            for g in range(4):
                s = slice(32 * g, 32 * g + 32)
                nc.gpsimd.dma_start(
                    out=w_all[s].rearrange("p co kh kw -> p (co kh kw)"),
                    in_=dram_ap(w_d, 0, [(288, 32), (1, 288)]),
                )
                nc.gpsimd.dma_start(
                    out=waw_all[s], in_=dram_ap(waw_d, 0, [(512, 32), (1, 512)])
                )
                nc.gpsimd.dma_start(
                    out=baw_all[s], in_=dram_ap(baw_d, 0, [(512, 32), (1, 512)])
                )
                nc.gpsimd.dma_start(
                    out=bias_all[s], in_=dram_ap(bias_d, 0, [(1, 32), (1, 1)])
                )
                nc.gpsimd.dma_start(
                    out=wab_all[s], in_=dram_ap(wab_d, 0, [(1, 32), (1, 1)])
                )
                nc.gpsimd.dma_start(
                    out=bab_all[s], in_=dram_ap(bab_d, 0, [(1, 32), (1, 1)])
                )

            # ---- per-sample adapters: wadapt/badapt, modulated weights ----
            wmod = []
            biasvec = []
            for b in range(SB):
                ibt = const.tile([128, 2 * EMB], f32, tag=f"ib{b}")
                nc.gpsimd.dma_start(
                    out=ibt[:, 0:EMB], in_=dram_ap(cond_d, b * EMB, [(0, 128), (1, EMB)])
                )
                nc.gpsimd.dma_start(
                    out=ibt[:, EMB : 2 * EMB],
                    in_=dram_ap(lpe_d, b * EMB, [(0, 128), (1, EMB)]),
                )
                scr_a = const.tile([128, 2 * EMB], f32, tag=f"scr_a{b}")
                wad = const.tile([128, 1], f32, tag=f"wad{b}")
                nc.vector.tensor_mul(scr_a, waw_all, ibt)
                nc.vector.reduce_sum(wad, scr_a, axis=mybir.AxisListType.X)
                nc.vector.tensor_add(wad, wad, wab_all)
                scr_b = const.tile([128, 2 * EMB], f32, tag=f"scr_b{b}")
                bad = const.tile([128, 1], f32, tag=f"bad{b}")
                nc.vector.tensor_mul(scr_b, baw_all, ibt)
                nc.vector.reduce_sum(bad, scr_b, axis=mybir.AxisListType.X)
                nc.vector.tensor_add(bad, bad, bab_all)
                wm = const.tile([128, COUT, KK, KK], bf16, tag=f"wmod{b}")
                nc.vector.tensor_scalar_mul(
                    wm.rearrange("p co kh kw -> p (co kh kw)"),
                    w_all.rearrange("p co kh kw -> p (co kh kw)"),
                    wad,
                )
                bv = const.tile([128, 1], f32, tag=f"bv{b}")
                nc.vector.tensor_scalar_mul(bv, bias_all, bad)
                wmod.append(wm)
                biasvec.append(bv)

            # dw=0 taps first so the start=True matmul covers all 512 columns;
            # kh=1 first so the start=True matmul never needs edge-row trimming
            TAPS = [(kh, dw) for dw in (0, -1, 1) for kh in (1, 0, 2)]

            # ---- main conv loop ----
            def conv_body():
              for b in range(n_samples):
                for Ti in range(n_tiles):  # 128 output rows per tile
                    xt = xpool.tile([128, 34, 256], f32, tag="xt")
                    xoff = b * SOFF
                    # quarter g holds image rows [base, base+34) (1-row halo
                    # each side), clipped at image edges; out-of-image halo
                    # rows stay uninitialized — the taps reading them are
                    # trimmed below
                    for g in range(4):
                        base = 128 * Ti + 32 * g - 1
                        lo = max(base, 0)
                        hi = min(base + 34, H)
                        if lo > base:
                            nc.vector.memset(xt[32 * g : 32 * g + 32, 0 : lo - base, :], 0.0)
                        if hi < base + 34:
                            nc.vector.memset(
                                xt[32 * g : 32 * g + 32, hi - base : 34, :], 0.0
                            )
                        nc.sync.dma_start(
                            out=xt[32 * g : 32 * g + 32, lo - base : hi - base, :],
                            in_=dram_ap(
                                x_d, xoff + lo * W, [(HW_, 32), (W, hi - lo), (1, W)]
                            ),
                        )

                    # f32 -> bf16 conversion pass (DVE, full 128 partitions);
                    # split so early waves can start before the tail converts
                    xb = xpool.tile([128, 34, 256], bf16, tag="xb")
                    chunks = ((0, 1),) if skip_conv else ((0, 12), (12, 20), (20, 28), (28, 34))
                    for lo, hi in chunks:
                        nc.vector.tensor_copy(
                            xb[:, lo:hi, :].rearrange("p r w -> p (r w)"),
                            xt[:, lo:hi, :].rearrange("p r w -> p (r w)"),
                        )

                    for wv in range(n_waves):  # 32 output rows per wave (8/quarter)
                        ps = ppool.tile([128, 4, 2, 256], f32, tag="ps")
                        for ti, (kh, dw) in enumerate(TAPS[:n_taps]):
                            last = ti == n_taps - 1
                            for q in range(n_q):
                                lhsT = wmod[b][32 * q : 32 * q + 32, :, kh, dw + 1]
                                for j in range(n_j):
                                    jp = slice(32 * j, 32 * j + 32)
                                    qp = slice(32 * q, 32 * q + 32)
                                    rb = 8 * wv + 2 * j + kh
                                    # absolute input row for r2=0/1; trim rows
                                    # outside the image (SAME zero padding)
                                    r0 = 128 * Ti + 32 * q + 8 * wv + 2 * j + kh - 1
                                    r2lo = 1 if r0 < 0 else 0
                                    r2hi = 1 if r0 + 1 > H - 1 else 2
                                    if dw == 0:
                                        # rows contiguous -> one flat-N matmul
                                        nc.tensor.matmul(
                                            ps[jp, q, r2lo:r2hi, :].rearrange(
                                                "p a b -> p (a b)"
                                            ),
                                            lhsT,
                                            xb[qp, rb + r2lo : rb + r2hi, :].rearrange(
                                                "p a b -> p (a b)"
                                            ),
                                            start=(ti == 0),
                                            stop=last and r2hi == 0,
                                            tile_position=(32 * q, 32 * j),
                                            skip_group_check=True,
                                        )
                                    else:
                                        # w-shifted taps: per-row matmuls (2D APs)
                                        rc0, oc0 = (0, 1) if dw == -1 else (1, 0)
                                        for r2 in range(r2lo, r2hi):
                                            nc.tensor.matmul(
                                                ps[jp, q, r2, oc0 : oc0 + 255],
                                                lhsT,
                                                xb[qp, rb + r2, rc0 : rc0 + 255],
                                                start=False,
                                                stop=last and r2 == r2hi - 1,
                                                tile_position=(32 * q, 32 * j),
                                                skip_group_check=True,
                                            )
                        og = opool.tile([128, 2048], f32, tag="og")
                        nc.scalar.activation(
                            og[:],
                            ps.rearrange("p q r w -> p (q r w)"),
                            Identity,
                            bias=biasvec[b],
                            scale=1.0,
                        )
                        for jj in range(4):
                            nc.scalar.dma_start(
                                out=dram_ap(
                                    out_d,
                                    b * SOFF + (128 * Ti + 8 * wv + 2 * jj) * W,
                                    [(HW_, 32), (32 * W, 4), (1, 2 * W)],
                                ),
                                in_=og[32 * jj : 32 * jj + 32, :].rearrange(
                                    "co (qq rw) -> co qq rw", qq=4
                                ),
                            )

            if reps_loop > 1:
                with tc.For_i(0, reps_loop, 1):
                    conv_body()
            else:
                conv_body()

    nc.finalize()
    return nc


def _get_nc():
    if "nc" not in _CACHE:
        _CACHE["nc"] = _build_nc()
    return _CACHE["nc"]


def kernel(**inputs):
    from concourse.bass_utils import run_bass_kernel_spmd

    nc = _get_nc()
    res = _run(nc, inputs, run_bass_kernel_spmd, trace=False)
    return _gather(res)


def _run(nc, inputs, run_bass_kernel_spmd, trace=False, trace_kwargs=None):
    in_maps = []
    for c in range(NCORES):
        s = slice(c * SB, (c + 1) * SB)
        in_maps.append(
            {
                "x": np.ascontiguousarray(inputs["x"][s], dtype=np.float32),
                "condition": np.ascontiguousarray(
                    inputs["condition"][s], dtype=np.float32
                ),
                "layer_pos_embedding": np.ascontiguousarray(
                    inputs["layer_pos_embedding"][s], dtype=np.float32
                ),
                "weights": np.ascontiguousarray(inputs["weights"], dtype=np.float32),
                "bias": np.ascontiguousarray(inputs["bias"], dtype=np.float32),
                "wa_w": np.ascontiguousarray(inputs["wa_w"], dtype=np.float32),
                "wa_b": np.ascontiguousarray(inputs["wa_b"], dtype=np.float32),
                "ba_w": np.ascontiguousarray(inputs["ba_w"], dtype=np.float32),
                "ba_b": np.ascontiguousarray(inputs["ba_b"], dtype=np.float32),
            }
        )
    kwargs = {}
    if trace:
        kwargs["trace"] = True
        if trace_kwargs:
            kwargs["trace_kwargs"] = trace_kwargs
    return run_bass_kernel_spmd(nc, in_maps, core_ids=list(range(NCORES)), **kwargs)


def _gather(res):
    return np.concatenate([res.results[c]["out"] for c in range(NCORES)], axis=0)



# revision 7
# speedup vs baseline: 1.0250x; 1.0250x over previous
"""BatchAdaptiveConv2d Trainium2 kernel (8 NeuronCores, data parallel).

Math: out[b] = conv2d_same(x[b], W * wadapt[b, ci]) + bias * badapt[b]
 - wadapt[b] = cat(cond[b], lpe[b]) @ wa_w.T + wa_b    (per-sample Cin scale)
 - badapt[b] = cat(cond[b], lpe[b]) @ ba_w.T + ba_b    (per-sample Cout bias scale)

Per-core plan (2 samples each), high-K matmul formulation:
 - x tile [128 = (s, g, ci), 65, 258] f32: g in {0,1} are row-shifted
   duplicates (at tile row t, g=0 holds x row t-1, g=1 holds x row t),
   loaded by reading HBM twice; cols 0/257 are the SAME zero pad.
 - Each PSUM block covers 8 output rows (2 stripes of 4 phases p):
   9 matmuls (3 row-groups k x 3 kw shifts) of K=64=(g,ci),
   M=128=(p,co), N=512=(2 stripes, 256 w). Row-group k reads tile row
   4j+2k, supplying x rows {4j+2k-1, 4j+2k}; tap (p,kh) lands in
   row-group k=(p+kh)//2, g=(p+kh)%2 -- exactly once each.
 - lhsT slot (3k+kw): [64=(g,ci), (p,co)] = W[ci,co,2k+g-p,kw]*wadapt[s,ci]
   built on-chip into a zeroed [128, 9, 4, 32] table via DVE strided copies.
 - float32r matmuls (full rate at N=512, no bf16 conversion pass).
 - PSUM -> SBUF via ScalarE Identity-activation with per-partition bias
   (= bias*badapt replicated x4 over phase groups), then wide DMAs out.
"""

import numpy as np

B, CIN, COUT, KK, H, W = 16, 32, 32, 3, 256, 256
EMB = 256
NCORES = 8
SB = B // NCORES  # samples per core

_CACHE = {}

RT = 32  # output rows per x tile
NT = H // RT  # 4 row tiles
WR = W + 2  # padded row width


def _build_nc():
    import concourse.bacc as bacc
    import concourse.bass as bass
    import concourse.mybir as mybir
    from concourse.tile import TileContext

    f32 = mybir.dt.float32
    bf16 = mybir.dt.bfloat16
    Identity = mybir.ActivationFunctionType.Identity

    nc = bacc.Bacc()

    x_d = nc.declare_dram_parameter("x", [SB, CIN, H, W], f32, isOutput=False)
    cond_d = nc.declare_dram_parameter("condition", [SB, EMB], f32, isOutput=False)
    lpe_d = nc.declare_dram_parameter(
        "layer_pos_embedding", [SB, EMB], f32, isOutput=False
    )
    w_d = nc.declare_dram_parameter("weights", [CIN, COUT, KK, KK], f32, isOutput=False)
    bias_d = nc.declare_dram_parameter("bias", [COUT], f32, isOutput=False)
    waw_d = nc.declare_dram_parameter("wa_w", [CIN, 2 * EMB], f32, isOutput=False)
    wab_d = nc.declare_dram_parameter("wa_b", [CIN], f32, isOutput=False)
    baw_d = nc.declare_dram_parameter("ba_w", [COUT, 2 * EMB], f32, isOutput=False)
    bab_d = nc.declare_dram_parameter("ba_b", [COUT], f32, isOutput=False)
    out_d = nc.declare_dram_parameter("out", [SB, COUT, H, W], f32, isOutput=True)

    def dram_ap(handle, offset, dims):
        a = handle[:]
        return bass.AP(tensor=a.tensor, offset=offset, ap=[list(d) for d in dims])

    HW_ = H * W  # 65536, per-channel plane
    SOFF = CIN * HW_  # per-sample x offset
    XP = (RT + 1) * WR  # per-partition x-tile elements

    with TileContext(nc) as tc:
        with (
            tc.tile_pool(name="const", bufs=1) as const,
            tc.tile_pool(name="xin", bufs=2) as xpool,
            tc.tile_pool(name="ostg", bufs=3) as opool,
            tc.tile_pool(name="ps", bufs=4, space="PSUM") as ppool,
        ):
            # ---- constants replicated x4 across partition quarter groups ----
            wrep = const.tile([128, COUT, KK, KK], f32, tag="wrep")
            nc.gpsimd.dma_start(
                out=wrep.rearrange("q co kh kw -> q (co kh kw)"),
                in_=dram_ap(w_d, 0, [(0, 4), (288, 32), (1, 288)]),
            )
            waw_all = const.tile([128, 2 * EMB], f32, tag="waw_all")
            nc.gpsimd.dma_start(
                out=waw_all, in_=dram_ap(waw_d, 0, [(0, 4), (512, 32), (1, 512)])
            )
            baw_all = const.tile([128, 2 * EMB], f32, tag="baw_all")
            nc.gpsimd.dma_start(
                out=baw_all, in_=dram_ap(baw_d, 0, [(0, 4), (512, 32), (1, 512)])
            )
            wab_all = const.tile([128, 1], f32, tag="wab_all")
            nc.gpsimd.dma_start(
                out=wab_all, in_=dram_ap(wab_d, 0, [(0, 4), (1, 32), (1, 1)])
            )
            bab_all = const.tile([128, 1], f32, tag="bab_all")
            nc.gpsimd.dma_start(
                out=bab_all, in_=dram_ap(bab_d, 0, [(0, 4), (1, 32), (1, 1)])
            )
            bias_all = const.tile([128, 1], f32, tag="bias_all")
            nc.gpsimd.dma_start(
                out=bias_all, in_=dram_ap(bias_d, 0, [(0, 4), (1, 32), (1, 1)])
            )

            # ib: [128=(s,g,ci), 512] = cat(cond[s], lpe[s]) per sample-half
            ib = const.tile([128, 2 * EMB], f32, tag="ib")
            for s in range(SB):
                nc.gpsimd.dma_start(
                    out=ib[64 * s : 64 * s + 64, 0:EMB],
                    in_=dram_ap(cond_d, s * EMB, [(0, 64), (1, EMB)]),
                )
                nc.gpsimd.dma_start(
                    out=ib[64 * s : 64 * s + 64, EMB : 2 * EMB],
                    in_=dram_ap(lpe_d, s * EMB, [(0, 64), (1, EMB)]),
                )

            # wadapt[(s,g,ci), 1] for both samples in one shot
            scr = const.tile([128, 2 * EMB], f32, tag="scr")
            wad = const.tile([128, 1], f32, tag="wad")
            nc.vector.tensor_mul(scr, waw_all, ib)
            nc.vector.reduce_sum(wad, scr, axis=mybir.AxisListType.X)
            nc.vector.tensor_add(wad, wad, wab_all)

            # modulated weights [(s,g,ci), co, kh, kw]
            wmod = const.tile([128, COUT, KK, KK], f32, tag="wmod")
            nc.vector.tensor_scalar_mul(
                wmod.rearrange("q co kh kw -> q (co kh kw)"),
                wrep.rearrange("q co kh kw -> q (co kh kw)"),
                wad,
            )

            # lhsT table [128=(s,g,ci), slot=3k+kw, p, co]:
            # slot holds wmod[.., co, 2k+g-p, kw] where 0<=2k+g-p<3, else 0
            lall = const.tile([128, 9, 4, 32], bf16, tag="lall")
            nc.vector.memset(lall.rearrange("q a b c -> q (a b c)"), 0.0)
            for s in range(SB):
                for g in range(2):
                    base = 64 * s + 32 * g
                    for k in range(3):
                        for p in range(4):
                            kh = 2 * k + g - p
                            if not (0 <= kh < KK):
                                continue
                            # dst [32, kw:3, 1, co:32] <- src transposed
                            nc.vector.tensor_copy(
                                lall[base : base + 32, 3 * k : 3 * k + 3, p : p + 1, :],
                                wmod[
                                    base : base + 32, :, kh : kh + 1, :
                                ].transpose([0, 3, 2, 1]),
                            )

            # per-sample ACT bias vec [(x4, co), 1] = bias[co] * badapt[s, co]
            bvs = []
            for s in range(SB):
                ib2s = const.tile([128, 2 * EMB], f32, tag=f"ib2_{s}")
                nc.gpsimd.dma_start(
                    out=ib2s[:, 0:EMB],
                    in_=dram_ap(cond_d, s * EMB, [(0, 128), (1, EMB)]),
                )
                nc.gpsimd.dma_start(
                    out=ib2s[:, EMB : 2 * EMB],
                    in_=dram_ap(lpe_d, s * EMB, [(0, 128), (1, EMB)]),
                )
                scr2 = const.tile([128, 2 * EMB], f32, tag=f"scr2_{s}")
                bad = const.tile([128, 1], f32, tag=f"bad{s}")
                nc.vector.tensor_mul(scr2, baw_all, ib2s)
                nc.vector.reduce_sum(bad, scr2, axis=mybir.AxisListType.X)
                nc.vector.tensor_add(bad, bad, bab_all)
                bv = const.tile([128, 1], f32, tag=f"bv{s}")
                nc.vector.tensor_mul(bv, bad, bias_all)
                bvs.append(bv)

            # ---- main loop over row tiles ----
            for t in range(NT):
                r0 = RT * t
                xg = xpool.tile([128, RT + 1, WR], f32, tag="xg")
                # zero pad columns 0 and 257
                nc.vector.memset(xg[:, :, 0:1], 0.0)
                nc.vector.memset(xg[:, :, W + 1 : W + 2], 0.0)
                # load: partition (s,g,ci) row tl holds x[s, ci, r0+tl+g-1]
                if t == 0:
                    for s in range(SB):
                        nc.vector.memset(xg[64 * s : 64 * s + 32, 0:1, 1 : W + 1], 0.0)
                        nc.sync.dma_start(
                            out=xg[64 * s : 64 * s + 32, 1 : RT + 1, 1 : W + 1],
                            in_=dram_ap(x_d, s * SOFF, [(HW_, 32), (W, RT), (1, W)]),
                        )
                        nc.sync.dma_start(
                            out=xg[64 * s + 32 : 64 * s + 64, :, 1 : W + 1],
                            in_=dram_ap(
                                x_d, s * SOFF, [(HW_, 32), (W, RT + 1), (1, W)]
                            ),
                        )
                elif t == NT - 1:
                    for s in range(SB):
                        nc.sync.dma_start(
                            out=xg[64 * s : 64 * s + 32, :, 1 : W + 1],
                            in_=dram_ap(
                                x_d,
                                s * SOFF + (r0 - 1) * W,
                                [(HW_, 32), (W, RT + 1), (1, W)],
                            ),
                        )
                        nc.sync.dma_start(
                            out=xg[64 * s + 32 : 64 * s + 64, 0:RT, 1 : W + 1],
                            in_=dram_ap(
                                x_d, s * SOFF + r0 * W, [(HW_, 32), (W, RT), (1, W)]
                            ),
                        )
                        nc.vector.memset(
                            xg[64 * s + 32 : 64 * s + 64, RT : RT + 1, 1 : W + 1], 0.0
                        )
                else:
                    for s in range(SB):
                        for g in range(2):
                            nc.sync.dma_start(
                                out=xg[
                                    64 * s + 32 * g : 64 * s + 32 * g + 32,
                                    :,
                                    1 : W + 1,
                                ],
                                in_=dram_ap(
                                    x_d,
                                    s * SOFF + (r0 + g - 1) * W,
                                    [(HW_, 32), (W, RT + 1), (1, W)],
                                ),
                            )

                # f32 -> bf16 conversion pass (full 128 partitions)
                xb = xpool.tile([128, RT + 1, WR], bf16, tag="xb")
                nc.vector.tensor_copy(
                    xb.rearrange("q r w -> q (r w)"),
                    xg.rearrange("q r w -> q (r w)"),
                )

                # blocks: 8 output rows each (2 stripes x 4 phases)
                ogs = [
                    opool.tile([128, 4, 2, 256], f32, tag=f"og{s}", name=f"og{s}")
                    for s in range(SB)
                ]
                for b2 in range(4):  # psum blocks within group
                    tl0 = 8 * b2
                    for s in range(SB):
                        ps = ppool.tile([128, 512], f32, tag="ps")
                        for k in range(3):
                            row = tl0 + 2 * k
                            for kw in range(3):
                                lhsT = lall[
                                    64 * s : 64 * s + 64,
                                    3 * k + kw : 3 * k + kw + 1,
                                    :,
                                    :,
                                ]
                                rhs = bass.AP(
                                    tensor=xb.tensor,
                                    offset=xb[64 * s : 64 * s + 64].offset
                                    + row * WR
                                    + kw,
                                    ap=[[XP, 64], [4 * WR, 2], [1, W]],
                                )
                                nc.tensor.matmul(
                                    ps[:],
                                    lhsT,
                                    rhs,
                                    start=(k == 0 and kw == 0),
                                    stop=(k == 2 and kw == 2),
                                )
                        nc.scalar.activation(
                            ogs[s][:, b2 : b2 + 1, :, :],
                            ps[:],
                            Identity,
                            bias=bvs[s],
                            scale=1.0,
                        )
                # store 32 rows per sample: 4 DMAs (one per phase p)
                for s in range(SB):
                    for p in range(4):
                        nc.scalar.dma_start(
                            out=dram_ap(
                                out_d,
                                s * SOFF + (r0 + p) * W,
                                [(HW_, 32), (8 * W, 4), (4 * W, 2), (1, W)],
                            ),
                            in_=ogs[s][32 * p : 32 * p + 32, :, :, :],
                        )

    nc.finalize()
    return nc


def _get_nc():
    if "nc" not in _CACHE:
        _CACHE["nc"] = _build_nc()
    return _CACHE["nc"]


def kernel(**inputs):
    from concourse.bass_utils import run_bass_kernel_spmd

    nc = _get_nc()
    res = _run(nc, inputs, run_bass_kernel_spmd, trace=False)
    return _gather(res)


def _run(nc, inputs, run_bass_kernel_spmd, trace=False, trace_kwargs=None):
    in_maps = []
    for c in range(NCORES):
        s = slice(c * SB, (c + 1) * SB)
        in_maps.append(
            {
                "x": np.ascontiguousarray(inputs["x"][s], dtype=np.float32),
                "condition": np.ascontiguousarray(
                    inputs["condition"][s], dtype=np.float32
                ),
                "layer_pos_embedding": np.ascontiguousarray(
                    inputs["layer_pos_embedding"][s], dtype=np.float32
                ),
                "weights": np.ascontiguousarray(inputs["weights"], dtype=np.float32),
                "bias": np.ascontiguousarray(inputs["bias"], dtype=np.float32),
                "wa_w": np.ascontiguousarray(inputs["wa_w"], dtype=np.float32),
                "wa_b": np.ascontiguousarray(inputs["wa_b"], dtype=np.float32),
                "ba_w": np.ascontiguousarray(inputs["ba_w"], dtype=np.float32),
                "ba_b": np.ascontiguousarray(inputs["ba_b"], dtype=np.float32),
            }
        )
    kwargs = {}
    if trace:
        kwargs["trace"] = True
        if trace_kwargs:
            kwargs["trace_kwargs"] = trace_kwargs
    return run_bass_kernel_spmd(nc, in_maps, core_ids=list(range(NCORES)), **kwargs)


def _gather(res):
    return np.concatenate([res.results[c]["out"] for c in range(NCORES)], axis=0)
